# revision 17
# baseline (speedup 1.0000x reference)
"""Trainium2 Bass kernel for: out = relu(einsum('bcs,cs->bs', x, w) + bias).

Full shapes: x [32, 2048, 4096] f32, w [2048, 4096] f32, bias [4096] f32.
Sharding: the s-axis (4096) is split across 8 cores (512 each). Each core
reads its x slice (128 MiB) and w/bias slice (4 MiB) once — the minimum
possible HBM traffic — and produces out[:, s_slice]. Gather = concat.

Per-core dataflow (partitions = 128-channel block, free = s):
  DMA   x[b] slice  -> SBUF [128, 16*512]          (4 MiB per batch)
  DVE   xb *= w     (fp32 elementwise, in place)
  PE    ones-matmul per c-block, accumulating the 128-partition reduction
        of each [128, 512] product block into PSUM [1, 512]; the bias row
        is folded in as a K=1 matmul that opens the accumulation group.
  ACT   relu during PSUM -> SBUF copy into out row b
  DMA   out [32, 512] -> DRAM
"""

import numpy as np

B, C, S_FULL = 32, 2048, 4096
N_CORES = 8
S = S_FULL // N_CORES          # 512 s-values per core
P = 128                        # SBUF partitions
CB = C // P                    # 16 channel blocks

# PE reduction dtype: fp32 matmul streams at 4 cyc/row; float32r at 1 cyc/row
# (reduced precision — validated empirically against the fp32 reference).
USE_F32R = True
# First K_FOLD block-pairs are pre-added on DVE to offload the (4x slower)
# fp32 PE reduction. 0 disables. Only meaningful when USE_F32R is False.
K_FOLD = 4

_nc_cache = {}


def _build():
    import concourse.bacc as bacc
    import concourse.mybir as mybir
    import concourse.tile as tile

    f32 = mybir.dt.float32
    nc = bacc.Bacc(
        "TRN2",
        target_bir_lowering=False,
        debug=False,
        enable_asserts=False,
        num_devices=N_CORES,
    )

    x = nc.dram_tensor("xs", [B, C, S], f32, kind="ExternalInput").ap()
    w = nc.dram_tensor("ws", [C, S], f32, kind="ExternalInput").ap()
    bias = nc.dram_tensor("bs", [1, S], f32, kind="ExternalInput").ap()
    out = nc.dram_tensor("out", [B, S], f32, kind="ExternalOutput").ap()

    with tile.TileContext(nc) as tc:
        with (
            tc.tile_pool(name="const", bufs=1) as cpool,
            tc.tile_pool(name="xp", bufs=2) as xpool,
            tc.tile_pool(name="pp", bufs=2) as ppool,
            tc.tile_pool(name="ps", bufs=4, space="PSUM") as pspool,
            tc.tile_pool(name="op", bufs=4) as opool,
        ):
            w_sb = cpool.tile([P, CB * S], f32)
            nc.sync.dma_start(
                w_sb[:].rearrange("p (cb s) -> p cb s", cb=CB),
                w.rearrange("(cb p) s -> p cb s", p=P),
            )

            # lhsT of the reduction matmuls; float32r requires every matmul
            # input be produced with float32r dtype (rounded on write).
            red_dt = mybir.dt.float32r if USE_F32R else f32
            ones_f32 = cpool.tile([P, 1], f32)
            nc.vector.memset(ones_f32[:], 1.0)
            if USE_F32R:
                # memset can't write float32r; round via DVE copy
                ones = cpool.tile([P, 1], red_dt)
                nc.vector.tensor_copy(ones[:], ones_f32[:])
            else:
                ones = ones_f32

            bias_sb = cpool.tile([1, S], f32)
            nc.sync.dma_start(bias_sb[:], bias[:])



            for b in range(B):
                xb = xpool.tile([P, CB * S], f32, tag="xb")
                nc.sync.dma_start(
                    xb[:].rearrange("p (cb s) -> p cb s", cb=CB),
                    x[b].rearrange("(cb p) s -> p cb s", p=P),
                )
                # x * w; under float32r the product is rounded on write so
                # the reduction matmuls may consume it. (A separate product
                # tile — the verifier's aliasing analysis rejects in-place
                # rounding between the x DMA and the fp32r matmul reads.)
                prod = ppool.tile([P, CB * S], red_dt, tag="prod")
                nc.vector.tensor_mul(prod[:], xb[:], w_sb[:])

                nfold = 0 if USE_F32R else K_FOLD
                for k in range(nfold):
                    # fold block 2k+1 into block 2k on DVE
                    dst = prod[:, 2 * k * S : (2 * k + 1) * S]
                    src = prod[:, (2 * k + 1) * S : (2 * k + 2) * S]
                    nc.vector.tensor_add(dst, dst, src)

                ps = pspool.tile([1, S], f32)
                # bias fold-in: K=1 matmul opens the accumulation group
                # (plain fp32 — 512 rows, negligible PE time)
                nc.tensor.matmul(
                    ps[:], ones_f32[0:1, 0:1], bias_sb[:], start=True, stop=False
                )
                blocks = [2 * k for k in range(nfold)] + list(range(2 * nfold, CB))
                for i, cb in enumerate(blocks):
                    rhs = prod[:, cb * S : (cb + 1) * S]
                    nc.tensor.matmul(
                        ps[:], ones[:], rhs, start=False, stop=(i == len(blocks) - 1)
                    )

                row = opool.tile([1, S], f32, tag="row")
                nc.scalar.activation(
                    row[:], ps[:], mybir.ActivationFunctionType.Relu
                )
                nc.sync.dma_start(out[b : b + 1, :], row[:])

    nc.compile()
    return nc


def _get_nc():
    if "nc" not in _nc_cache:
        _nc_cache["nc"] = _build()
    return _nc_cache["nc"]


def _shard_inputs(x, weights, bias):
    x = np.asarray(x)
    weights = np.asarray(weights)
    bias = np.asarray(bias)
    in_maps = []
    for i in range(N_CORES):
        sl = slice(i * S, (i + 1) * S)
        in_maps.append(
            {
                "xs": np.ascontiguousarray(x[:, :, sl], dtype=np.float32),
                "ws": np.ascontiguousarray(weights[:, sl], dtype=np.float32),
                "bs": np.ascontiguousarray(
                    bias[sl].reshape(1, S), dtype=np.float32
                ),
            }
        )
    return in_maps


def _run(inputs, trace=False, trace_cores=None):
    from concourse import bass_utils

    nc = _get_nc()
    in_maps = _shard_inputs(inputs["x"], inputs["weights"], inputs["bias"])
    res = bass_utils.run_bass_kernel_spmd(
        nc,
        in_maps,
        core_ids=list(range(N_CORES)),
        trace=trace,
        trace_cores=trace_cores,
    )
    out = np.concatenate([r["out"] for r in res.results], axis=1)
    return out, res


def kernel(x, weights, bias):
    out, _ = _run({"x": x, "weights": weights, "bias": bias})
    return out
